# revision 25
# baseline (speedup 1.0000x reference)
"""Trainium2 Bass kernel for nn_DiPaFBackbone (vq_codebook).

Self-contained: kernel(**inputs) takes the FULL numpy inputs, shards the batch
dim over 8 NeuronCores, runs one SPMD Bass/Tile kernel, gathers full outputs.
Returns (outputs [32,512,64] f32, logits [32,64,32,1024] f32).

Pipeline per core (4 batches):
  RevIN stats (ones-matmul) -> encoder GEMM with the RevIN correction folded
  in as an augmented contraction row (LN scale-invariance kills 1/sigma) ->
  LayerNorm -> fc1/fcm/fc2 in transposed [h,(v,d)] layout (bf16, ReLU fused
  into PSUM evacuation) -> fc2 emits z_pD=[d,(v,f)] which is directly the
  lhsT of the VQ distance GEMM (f32r, x2 prescaled) -> K=2 PSUM-init matmul
  preloads -z^2-c^2 so PSUM = logits -> DVE max/max_index top-5 -> batched
  softmax -> 5 indirect-DMA codebook row gathers -> scalar_tensor_tensor
  weighted accumulate onto relu(fuse GEMM) -> fusion LN with sigma folded into
  the normalize scale -> PE transpose -> decoder GEMM with mu preloaded into
  PSUM -> [t,v] output tiles -> HBM.
"""

import numpy as np
import ml_dtypes

import concourse.bacc as bacc
import concourse.bass as bass
import concourse.mybir as mybir
import concourse.tile as tile
from concourse.bass import IndirectOffsetOnAxis
from concourse.masks import make_identity

F32 = mybir.dt.float32
F32R = mybir.dt.float32r
BF16 = mybir.dt.bfloat16
U32 = mybir.dt.uint32
AF = mybir.ActivationFunctionType
OP = mybir.AluOpType
AX = mybir.AxisListType

B, L, V = 32, 1024, 64
PL, PH, PRED, PF = 16, 64, 512, 32
D, H, K, TOPK, TEMP, EPS = 128, 256, 1024, 5, 1.0, 1e-5
N_CORES = 8
B_LOC = B // N_CORES      # 4
NT = V * PF // 128        # 16 n-tiles per batch, n-tile = 4 channels x 32 steps
NG = V // 4               # 16 groups of 4 channels for the MLP phase


def build_nc(b_loc=B_LOC, repeat=1):
    nc = bacc.Bacc(None, target_bir_lowering=False)

    x_in = nc.dram_tensor("x_s", [b_loc, L, V], F32, kind="ExternalInput")
    cent = nc.dram_tensor("centroids", [K, D], F32, kind="ExternalInput")
    enc_rhs_d = nc.dram_tensor("enc_rhs", [17, D], BF16, kind="ExternalInput")
    fc1_lhs_d = nc.dram_tensor("fc1_lhs", [128, H], BF16, kind="ExternalInput")
    fc1_b_d = nc.dram_tensor("fc1_b2", [128, 2], F32, kind="ExternalInput")
    fcm_lhs_d = nc.dram_tensor("fcm_lhs", [128, 2, 2 * H], BF16, kind="ExternalInput")
    fcm_b_d = nc.dram_tensor("fcm_b4", [128, 4], F32, kind="ExternalInput")
    fc2_rhs_d = nc.dram_tensor("fc2_rhs", [128, 4, PF], BF16, kind="ExternalInput")
    ct_d = nc.dram_tensor("ct", [D, K], F32R, kind="ExternalInput")
    c2aug_d = nc.dram_tensor("c2aug", [2, K], F32R, kind="ExternalInput")
    fuse_rhs_d = nc.dram_tensor("fuse_rhs", [D, D], F32R, kind="ExternalInput")
    dec_lhs_d = nc.dram_tensor("dec_lhs", [D, PL], BF16, kind="ExternalInput")

    out_s = nc.dram_tensor("out_s", [b_loc, PRED, V], F32, kind="ExternalOutput")
    logit_s = nc.dram_tensor("logit_s", [b_loc, V, PF, K], F32, kind="ExternalOutput")

    with tile.TileContext(nc) as tc:
        with (
            tc.tile_pool(name="const", bufs=1) as cpool,
            tc.tile_pool(name="work", bufs=2) as wpool,
            tc.tile_pool(name="mlp", bufs=4) as mpool,
            tc.tile_pool(name="vq", bufs=3) as vpool,
            tc.tile_pool(name="psE", bufs=2, space="PSUM") as psE,
            tc.tile_pool(name="psB", bufs=3, space="PSUM") as psB,
            tc.tile_pool(name="psS", bufs=3, space="PSUM") as psS,
            tc.tile_pool(name="dram", bufs=2, space="DRAM") as dpool,
        ):
            # ------------- constants -------------
            enc_rhs = cpool.tile_from(enc_rhs_d[:])
            fc1_lhs = cpool.tile_from(fc1_lhs_d[:])
            fc1_b = cpool.tile_from(fc1_b_d[:])
            fcm_lhs = cpool.tile_from(fcm_lhs_d[:])
            fcm_b = cpool.tile_from(fcm_b_d[:])
            fc2_rhs = cpool.tile_from(fc2_rhs_d[:])
            ct = cpool.tile_from(ct_d[:])
            c2aug = cpool.tile_from(c2aug_d[:])
            fuse_rhs = cpool.tile_from(fuse_rhs_d[:])
            dec_lhs = cpool.tile_from(dec_lhs_d[:])

            ident_bf = cpool.tile([128, 128], BF16)
            make_identity(nc, ident_bf[:])
            ident_f = cpool.tile([128, 128], F32)
            make_identity(nc, ident_f[:])
            ones_col_f = cpool.tile([128, 1], F32)
            nc.vector.memset(ones_col_f[:], 1.0)
            ones_col_bf = cpool.tile([128, 1], BF16)
            nc.vector.memset(ones_col_bf[:], 1.0)
            ones_row_bf = cpool.tile([1, 16], BF16)
            nc.vector.memset(ones_row_bf[:], 1.0)
            ones_sq = cpool.tile([64, 32], F32)
            nc.vector.memset(ones_sq[:], 1.0)
            ones_row64 = cpool.tile([1, 64], F32)
            nc.vector.memset(ones_row64[:], 1.0)
            zero_col = cpool.tile([128, 1], F32)
            nc.vector.memset(zero_col[:], 0.0)
            eps_col = cpool.tile([128, 1], F32)
            nc.vector.memset(eps_col[:], EPS)
            nc.const_aps.aps[(F32, 0.0)] = zero_col[:]
            nc.const_aps.aps[(F32, EPS)] = eps_col[:]

            for rep in range(repeat):
              for b in range(b_loc):
                # ==================================================
                # S1 RevIN stats: sum / sumsq over L via ones-matmul
                # ==================================================
                pstat = psS.tile([1, 512], F32, tag="s")
                for i in range(8):
                    xst = wpool.tile([128, 128], F32, tag="xst", bufs=3)
                    nc.sync.dma_start(xst[:, 0:64], x_in[b, i * 128:(i + 1) * 128, :])
                    nc.scalar.square(xst[:, 64:128], xst[:, 0:64])
                    nc.tensor.matmul(pstat[:, 0:128], ones_col_f[:], xst[:],
                                     start=(i == 0), stop=(i == 7))
                # srow: [mu(64) | sigma(64) | scratch(64)]
                srow = wpool.tile([1, 192], F32, tag="srow")
                nc.vector.tensor_scalar_mul(srow[:, 0:64], pstat[:, 0:64], 1.0 / L)
                # var = E[x^2] - mu^2 : scratch = -mu*mu, then += E[x2]
                nc.vector.scalar_tensor_tensor(
                    out=srow[:, 128:192], in0=srow[:, 0:64], scalar=-1.0,
                    in1=srow[:, 0:64], op0=OP.mult, op1=OP.mult)
                nc.vector.scalar_tensor_tensor(
                    out=srow[:, 128:192], in0=pstat[:, 64:128], scalar=1.0 / L,
                    in1=srow[:, 128:192], op0=OP.mult, op1=OP.add)
                nc.scalar.activation(srow[:, 64:128], srow[:, 128:192],
                                     AF.Sqrt, bias=EPS)
                # bf16 [mu | sigma] row for decoder psum-init
                musig_bf = wpool.tile([1, 128], BF16, tag="musig_bf")
                nc.vector.tensor_copy(musig_bf[:], srow[:, 0:128])
                # [mu|sigma] as columns via PE transpose
                pcol = psS.tile([128, 512], F32, tag="s")
                nc.tensor.transpose(pcol[:, 0:1], srow[:, 0:128], ident_f[0:1, 0:1])
                musig_col = wpool.tile([128, 1], F32, tag="musig_col")
                nc.vector.tensor_copy(musig_col[:], pcol[:, 0:1])
                sig_col = musig_col[64:128, :]

                # -mu replicated: [64 rep, 64 v] bf16 -> DRAM (outer product)
                pneg = psS.tile([128, 512], F32, tag="s")
                nc.tensor.matmul(pneg[0:64, 0:64], ones_row64[:],
                                 srow[:, 0:64], start=True, stop=True)
                negmu_rep = wpool.tile([64, 64], BF16, tag="negmu_rep")
                nc.vector.tensor_scalar(
                    out=negmu_rep[:], in0=pneg[0:64, 0:64],
                    scalar1=-1.0, scalar2=None, op0=OP.mult)
                negmu_d = dpool.tile([64, 64], BF16, tag="negmu_d")
                nc.sync.dma_start(negmu_d[:], negmu_rep[:])
                # sigma replicated per (v,f): [64 v, 32 f] f32 -> DRAM
                srep_sb = wpool.tile([64, 32], F32, tag="srep_sb")
                nc.vector.tensor_scalar_mul(srep_sb[:], ones_sq[:], sig_col)
                srep_d = dpool.tile([64, 32], F32, tag="srep_d")
                nc.sync.dma_start(srep_d[:], srep_sb[:])

                # ==================================================
                # S2 patch lhsT [17, (p v)] bf16; row16 = -mu_v
                # ==================================================
                xpatch = wpool.tile([17, PH * V], BF16, tag="xpatch")
                xsrc = x_in[b].rearrange("(p l) v -> l p v", l=PL)
                for q in range(4):
                    xpf = wpool.tile([16, PH * V // 4], F32, tag="xpatch_f",
                                     bufs=2)
                    nc.sync.dma_start(
                        xpf[:].rearrange("l (p v) -> l p v", v=V),
                        xsrc[:, q * 16:(q + 1) * 16, :])
                    nc.vector.tensor_copy(
                        xpatch[0:16, q * 1024:(q + 1) * 1024], xpf[:])
                nc.sync.dma_start(xpatch[16:17, :],
                                  negmu_d[:].rearrange("r v -> (r v)")[None, :])

                # ==================================================
                # S3 encoder GEMM + bn_stats + evac y (pre-LN, bf16)
                # ==================================================
                # paired layout: group 2*gp+h lives on partitions h*64..h*64+64
                sums_b = wpool.tile([128, 32], F32, tag="sums_b")
                sq_b = wpool.tile([128, 32], F32, tag="sq_b")
                y_all = mpool.tile([128, NG // 2, 512], BF16, tag="y_all", bufs=1)
                for gp in range(NG // 2):
                    pe = psB.tile([128, 512], F32, tag="b")
                    for h in range(2):
                        g = 2 * gp + h
                        for vq in range(4):
                            v = g * 4 + vq
                            nc.tensor.matmul(
                                pe[h * 64:(h + 1) * 64, vq * 128:(vq + 1) * 128],
                                xpatch[:, v::V], enc_rhs[:],
                                start=True, stop=True)
                    nc.vector.tensor_reduce(
                        sums_b[:, gp * 4:(gp + 1) * 4],
                        pe[:].rearrange("p (c d) -> p c d", d=128),
                        axis=AX.X, op=OP.add)
                    nc.scalar.copy(y_all[:, gp, :], pe[:])
                    y2 = wpool.tile([128, 512], BF16, tag="y2", bufs=3)
                    nc.scalar.square(y2[:], pe[:])
                    nc.vector.tensor_reduce(
                        sq_b[:, gp * 4:(gp + 1) * 4],
                        y2[:].rearrange("p (c d) -> p c d", d=128),
                        axis=AX.X, op=OP.add)

                # batched LN scalars as [128, 32] (paired columns)
                ms_b = wpool.tile([128, 128], F32, tag="ms_b")
                mean_b, rstd_b = ms_b[:, 0:32], ms_b[:, 32:64]
                mr_b, tmp_b = ms_b[:, 64:96], ms_b[:, 96:128]
                nc.vector.tensor_scalar_mul(mean_b, sums_b[:], 1.0 / D)
                # var = sq/D - mean^2 ; rstd = 1/sqrt(var + 1e-5)
                nc.vector.scalar_tensor_tensor(out=mr_b, in0=mean_b, scalar=-1.0,
                                               in1=mean_b, op0=OP.mult, op1=OP.mult)
                nc.vector.scalar_tensor_tensor(out=tmp_b, in0=sq_b[:],
                                               scalar=1.0 / D,
                                               in1=mr_b, op0=OP.mult, op1=OP.add)
                nc.scalar.activation(tmp_b, tmp_b, AF.Sqrt, bias=1e-5)
                nc.vector.reciprocal(rstd_b, tmp_b)
                nc.vector.scalar_tensor_tensor(out=mr_b, in0=mean_b, scalar=-1.0,
                                               in1=rstd_b, op0=OP.mult, op1=OP.mult)

                # ==================================================
                # S4 LN normalize per channel: X0a = y*rstd - mean*rstd
                # ==================================================
                x0a = mpool.tile([128, NG // 2, 512], BF16, tag="x0a", bufs=1)
                engines = [nc.vector, nc.scalar]
                for v in range(V):
                    gp, h, vq = v // 8, (v // 4) % 2, v % 4
                    hs = slice(h * 64, (h + 1) * 64)
                    ysl = y_all[hs, gp, vq * 128:(vq + 1) * 128]
                    xsl = x0a[hs, gp, vq * 128:(vq + 1) * 128]
                    sc = rstd_b[hs, gp * 4 + vq:gp * 4 + vq + 1]
                    bi = mr_b[hs, gp * 4 + vq:gp * 4 + vq + 1]
                    eng = engines[v % 2]
                    if eng is nc.scalar:
                        nc.scalar.activation(xsl, ysl, AF.Identity,
                                             scale=sc, bias=bi)
                    else:
                        eng.scalar_tensor_tensor(
                            out=xsl, in0=ysl, scalar=sc,
                            in1=bi.to_broadcast([64, 128]),
                            op0=OP.mult, op1=OP.add)

                # ==================================================
                # S5 MLP: fc1 -> fcm -> fc2, z_pD2 = 2*z_p [d, (v f)]
                # ==================================================
                zpd2 = vpool.tile([128, NT * 128], F32R, tag="zpd2", bufs=2)
                zsq = vpool.tile([128, NT * 128], BF16, tag="zsq", bufs=1)
                for g in range(NG):
                    gp, h = g // 2, g % 2
                    hs = slice(h * 64, (h + 1) * 64)
                    o1 = []
                    for j in range(2):
                        p1 = psB.tile([128, 512], F32, tag="b")
                        nc.tensor.matmul(p1[:], fc1_lhs[hs, j * 128:(j + 1) * 128],
                                         x0a[hs, gp, :], start=True, stop=True)
                        t1 = mpool.tile([128, 512], BF16, tag="o1", bufs=4)
                        nc.scalar.activation(t1[:], p1[:], AF.Relu,
                                             bias=fc1_b[:, j:j + 1])
                        o1.append(t1)
                    o2 = []
                    for j2 in range(4):
                        p2 = psB.tile([128, 512], F32, tag="b")
                        for kt in range(2):
                            nc.tensor.matmul(
                                p2[:],
                                fcm_lhs[:, kt, j2 * 128:(j2 + 1) * 128],
                                o1[kt][:], start=(kt == 0), stop=(kt == 1))
                        t2 = mpool.tile([128, 512], BF16, tag="o2", bufs=8)
                        if j2 % 2 == 0:
                            nc.scalar.activation(t2[:], p2[:], AF.Relu,
                                                 bias=fcm_b[:, j2:j2 + 1])
                        else:
                            nc.vector.tensor_scalar(
                                out=t2[:], in0=p2[:], scalar1=fcm_b[:, j2:j2 + 1],
                                scalar2=0.0, op0=OP.add, op1=OP.max)
                        o2.append(t2)
                    pz = psS.tile([128, 512], F32, tag="s")
                    for vq in range(4):
                        v = g * 4 + vq
                        for kt in range(4):
                            nc.tensor.matmul(
                                pz[:, vq * 32:(vq + 1) * 32],
                                o2[kt][:, vq * 128:(vq + 1) * 128],
                                fc2_rhs[:, kt, :], start=(kt == 0), stop=(kt == 3))
                    # evac: z_pD2 = 2*z_p (f32r) ; zsq = (2 z_p)^2 bf16
                    nc.scalar.activation(zpd2[:, g * 128:(g + 1) * 128],
                                         pz[:, 0:128], AF.Copy, scale=2.0)
                    nc.scalar.square(zsq[:, g * 128:(g + 1) * 128],
                                     zpd2[:, g * 128:(g + 1) * 128])

                # z2 row: z2aug[0,:] = -0.25 * ones^T @ zsq = -|z_p|^2
                z2aug = vpool.tile([2, NT * 128], F32R, tag="z2aug", bufs=1)
                nc.vector.memset(z2aug[:, :].bitcast(F32), 1.0)
                for q in range(4):
                    pq = psS.tile([1, 512], F32, tag="s")
                    nc.tensor.matmul(pq[:], ones_col_bf[:],
                                     zsq[:, q * 512:(q + 1) * 512],
                                     start=True, stop=True)
                    nc.vector.tensor_scalar_mul(
                        z2aug[0:1, q * 512:(q + 1) * 512], pq[:], -0.25)

                # ==================================================
                # S6 VQ distances + logits out + top-5
                # ==================================================
                maxv_b = vpool.tile([128, NT * 8], F32, tag="maxv", bufs=2)
                maxi_b = vpool.tile([128, NT * 8], U32, tag="maxi", bufs=2)
                logits_t = []
                for t in range(NT):
                    lg = vpool.tile([128, K], F32, tag="lg", bufs=3)
                    for c in range(2):
                        pd = psB.tile([128, 512], F32, tag="b")
                        nc.tensor.matmul(
                            pd[:], z2aug[:, t * 128:(t + 1) * 128],
                            c2aug[:, c * 512:(c + 1) * 512],
                            start=True, stop=False)
                        nc.tensor.matmul(
                            pd[:], zpd2[:, t * 128:(t + 1) * 128],
                            ct[:, c * 512:(c + 1) * 512],
                            start=False, stop=True)
                        nc.scalar.activation(lg[:, c * 512:(c + 1) * 512], pd[:],
                                             AF.Copy, scale=1.0 / TEMP)
                    nc.sync.dma_start(
                        logit_s[b, 4 * t:4 * t + 4].rearrange("v f k -> (v f) k"),
                        lg[:])
                    nc.vector.max(out=maxv_b[:, t * 8:(t + 1) * 8], in_=lg[:])
                    nc.vector.max_index(out=maxi_b[:, t * 8:(t + 1) * 8],
                                        in_max=maxv_b[:, t * 8:(t + 1) * 8],
                                        in_values=lg[:])

                # batched top-5 softmax -> w5_b [128, t, 5] f32
                w5_b = vpool.tile([128, NT * 8], F32, tag="w5", bufs=2)
                den_b = vpool.tile([128, 2 * NT], F32, tag="den", bufs=2)
                mv = maxv_b[:].rearrange("p (t k) -> p t k", k=8)
                w5v = w5_b[:].rearrange("p (t k) -> p t k", k=8)
                nc.vector.tensor_tensor(
                    w5v[:, :, 0:5], mv[:, :, 0:5],
                    mv[:, :, 0:1].to_broadcast([128, NT, 5]), op=OP.subtract)
                nc.scalar.activation(w5v[:, :, 0:5], w5v[:, :, 0:5], AF.Exp)
                nc.vector.tensor_reduce(den_b[:, 0:NT], w5v[:, :, 0:5],
                                        axis=AX.X, op=OP.add)
                nc.vector.reciprocal(den_b[:, NT:2 * NT], den_b[:, 0:NT])
                nc.vector.tensor_tensor(
                    w5v[:, :, 0:5], w5v[:, :, 0:5],
                    den_b[:, NT:2 * NT, None].to_broadcast([128, NT, 5]),
                    op=OP.mult)

                # ==================================================
                # S7 fuse + gather + weighted sum + fusion LN stats
                # ==================================================
                s6f_b = vpool.tile([128, NT * 2], F32, tag="s6f", bufs=2)
                acc_b = vpool.tile([128, NT * 128], F32, tag="acc", bufs=1)
                for t in range(NT):
                    acc = acc_b[:, t * 128:(t + 1) * 128]
                    pf = psS.tile([128, 512], F32, tag="s")
                    nc.tensor.matmul(pf[:, 0:128],
                                     zpd2[:, t * 128:(t + 1) * 128],
                                     fuse_rhs[:], start=True, stop=True)
                    nc.scalar.activation(acc, pf[:, 0:128], AF.Relu, scale=0.5)
                    for slot in range(TOPK):
                        gt = vpool.tile([128, 128], F32, tag="gath", bufs=10)
                        nc.gpsimd.indirect_dma_start(
                            out=gt[:], out_offset=None, in_=cent[:],
                            in_offset=IndirectOffsetOnAxis(
                                ap=maxi_b[:, t * 8 + slot:t * 8 + slot + 1],
                                axis=0))
                        nc.vector.scalar_tensor_tensor(
                            out=acc, in0=gt[:],
                            scalar=w5_b[:, t * 8 + slot:t * 8 + slot + 1],
                            in1=acc, op0=OP.mult, op1=OP.add)
                    s6f = wpool.tile([128, 6], F32, tag="s6f_t", bufs=4)
                    nc.vector.bn_stats(s6f[:], acc)
                    nc.vector.bn_aggr(
                        s6f_b[:, t * 2:(t + 1) * 2], s6f[:])

                # batched fusion-LN scalars; fold sigma into scale
                srep_b = wpool.tile([128, NT], F32, tag="srep_b")
                nc.sync.dma_start(
                    srep_b[:], srep_d[:].rearrange("(t a) f -> (a f) t", a=4))
                msf_b = wpool.tile([128, 2 * NT], F32, tag="msf_b")
                s6fv = s6f_b[:].rearrange("p (t s) -> p t s", s=2)
                fsc_b, fbi_b = msf_b[:, 0:NT], msf_b[:, NT:2 * NT]
                nc.scalar.activation(fsc_b, s6fv[:, :, 1], AF.Sqrt, bias=1e-5)
                nc.vector.reciprocal(fsc_b, fsc_b)
                nc.vector.tensor_tensor(fsc_b, fsc_b, srep_b[:], op=OP.mult)
                nc.vector.scalar_tensor_tensor(out=fbi_b, in0=s6fv[:, :, 0],
                                               scalar=-1.0, in1=fsc_b,
                                               op0=OP.mult, op1=OP.mult)

                # ==================================================
                # S8 fusion-LN normalize (bf16) + transpose -> zfT
                # ==================================================
                zft = vpool.tile([128, NT * 128], BF16, tag="zft", bufs=1)
                for t in range(NT):
                    zfs = wpool.tile([128, 128], BF16, tag="zfs", bufs=4)
                    nc.scalar.activation(zfs[:], acc_b[:, t * 128:(t + 1) * 128],
                                         AF.Identity,
                                         scale=fsc_b[:, t:t + 1],
                                         bias=fbi_b[:, t:t + 1])
                    pt = psS.tile([128, 512], F32, tag="s")
                    ptb = pt[:, 0:64].bitcast(BF16)
                    nc.tensor.transpose(ptb, zfs[:], ident_bf[:])
                    nc.vector.tensor_copy(zft[:, t * 128:(t + 1) * 128], ptb)

                # ==================================================
                # S9 decoder: psum preloaded with mu, [t, v] layout out
                # ==================================================
                prt = psB.tile([128, 512], F32, tag="b")
                for f in range(PF):
                    sec = prt[0:64, f * 16:(f + 1) * 16]
                    nc.tensor.matmul(sec, musig_bf[:, 0:64], ones_row_bf[:],
                                     start=True, stop=False)
                    nc.tensor.matmul(sec, zft[:, f::PF], dec_lhs[:],
                                     start=False, stop=True)
                recT = wpool.tile([64, 512], F32, tag="recT", bufs=2)
                nc.scalar.copy(recT[:], prt[0:64, :])
                for blk in range(4):
                    pdc = psS.tile([128, 512], F32, tag="s")
                    nc.tensor.transpose(pdc[:, 0:64],
                                        recT[:, blk * 128:(blk + 1) * 128],
                                        ident_f[0:64, 0:64])
                    rec = wpool.tile([128, 64], F32, tag="rec", bufs=3)
                    nc.scalar.copy(rec[:], pdc[:, 0:64])
                    nc.sync.dma_start(
                        out_s[b, blk * 128:(blk + 1) * 128, :], rec[:])

    nc.compile()
    nc.compile()
    return nc


def _prep_weights(ip):
    """Host-side weight prep (shared across cores)."""
    bf = ml_dtypes.bfloat16
    enc_w = ip["enc_w"].astype(np.float32)          # [16, 128]
    enc_b = ip["enc_b"].astype(np.float32)          # [128]
    assert np.abs(enc_b).max() == 0.0, "enc_b fold not emitted"
    s_w = enc_w.sum(axis=0)                          # [128]
    enc_rhs = np.concatenate([enc_w, s_w[None, :]], 0).astype(bf)   # [17,128]

    ln_w, ln_b = ip["ln_w"], ip["ln_b"]
    assert np.allclose(ln_w, 1.0) and np.abs(ln_b).max() == 0.0, \
        "general ln_w/ln_b path not emitted"

    fc1_lhs = np.vstack([ip["fc1_w"], ip["fc1_w"]]).astype(bf)  # [128,256]
    fc1_b2 = np.ascontiguousarray(
        ip["fc1_b"].astype(np.float32).reshape(2, 128).T)   # [128, 2]
    fcm_lhs = np.ascontiguousarray(
        ip["fcm_w"].astype(bf).reshape(2, 128, 512).transpose(1, 0, 2))
    fcm_b4 = np.ascontiguousarray(
        ip["fcm_b"].astype(np.float32).reshape(4, 128).T)   # [128, 4]
    fc2_rhs = np.ascontiguousarray(
        ip["fc2_w"].astype(bf).reshape(4, 128, PF).transpose(1, 0, 2))
    assert np.abs(ip["fc2_b"]).max() == 0.0, "fc2_b fold not emitted"

    cent = ip["centroids"].astype(np.float32)        # [1024, 128]
    ct = np.ascontiguousarray(cent.T)                # [128, 1024]
    c2 = (cent.astype(np.float64) ** 2).sum(1).astype(np.float32)
    c2aug = np.stack([np.ones(K, np.float32), -c2])  # [2, 1024]

    fuse_rhs = ip["fuse_w"].astype(np.float32)       # [128, 128]
    assert np.abs(ip["fuse_b"]).max() == 0.0, "fuse_b fold not emitted"
    assert np.allclose(ip["fln_w"], 1.0) and np.abs(ip["fln_b"]).max() == 0.0, \
        "general fln path not emitted"
    dec_lhs = ip["dec_w"].astype(bf)                 # [128, 16]
    assert np.abs(ip["dec_b"]).max() == 0.0, "dec_b fold not emitted"

    return {
        "centroids": cent, "enc_rhs": enc_rhs, "fc1_lhs": fc1_lhs,
        "fc1_b2": fc1_b2, "fcm_lhs": fcm_lhs, "fcm_b4": fcm_b4,
        "fc2_rhs": fc2_rhs, "ct": ct, "c2aug": c2aug,
        "fuse_rhs": fuse_rhs, "dec_lhs": dec_lhs,
    }


_NC_CACHE = {}


def kernel(**inputs):
    from concourse.bass_utils import run_bass_kernel_spmd

    if "nc" not in _NC_CACHE:
        _NC_CACHE["nc"] = build_nc()
    nc = _NC_CACHE["nc"]

    w = _prep_weights(inputs)
    x = np.ascontiguousarray(inputs["x"].astype(np.float32))
    in_maps = []
    for c in range(N_CORES):
        m = dict(w)
        m["x_s"] = np.ascontiguousarray(x[c * B_LOC:(c + 1) * B_LOC])
        in_maps.append(m)

    res = run_bass_kernel_spmd(nc, in_maps, core_ids=list(range(N_CORES)))
    outs = np.concatenate([r["out_s"] for r in res.results], axis=0)
    logits = np.concatenate([r["logit_s"] for r in res.results], axis=0)
    return outs, logits


# revision 26
# speedup vs baseline: 1494.8080x; 1494.8080x over previous
"""Trainium2 Bass kernel for nn_DiPaFBackbone (vq_codebook).

Self-contained: kernel(**inputs) takes the FULL numpy inputs, shards the batch
dim over 8 NeuronCores, runs one SPMD Bass/Tile kernel, gathers full outputs.
Returns (outputs [32,512,64] f32, logits [32,64,32,1024] f32).

Pipeline per core (4 batches):
  RevIN stats (ones-matmul) -> encoder GEMM with the RevIN correction folded
  in as an augmented contraction row (LN scale-invariance kills 1/sigma) ->
  LayerNorm -> fc1/fcm/fc2 in transposed [h,(v,d)] layout (bf16, ReLU fused
  into PSUM evacuation) -> fc2 emits z_pD=[d,(v,f)] which is directly the
  lhsT of the VQ distance GEMM (f32r, x2 prescaled) -> K=2 PSUM-init matmul
  preloads -z^2-c^2 so PSUM = logits -> DVE max/max_index top-5 -> batched
  softmax -> 5 indirect-DMA codebook row gathers -> scalar_tensor_tensor
  weighted accumulate onto relu(fuse GEMM) -> fusion LN with sigma folded into
  the normalize scale -> PE transpose -> decoder GEMM with mu preloaded into
  PSUM -> [t,v] output tiles -> HBM.
"""

import numpy as np
import ml_dtypes

import concourse.bacc as bacc
import concourse.bass as bass
import concourse.mybir as mybir
import concourse.tile as tile
from concourse.bass import IndirectOffsetOnAxis
from concourse.masks import make_identity

F32 = mybir.dt.float32
F32R = mybir.dt.float32r
BF16 = mybir.dt.bfloat16
U32 = mybir.dt.uint32
AF = mybir.ActivationFunctionType
OP = mybir.AluOpType
AX = mybir.AxisListType

B, L, V = 32, 1024, 64
PL, PH, PRED, PF = 16, 64, 512, 32
D, H, K, TOPK, TEMP, EPS = 128, 256, 1024, 5, 1.0, 1e-5
N_CORES = 8
B_LOC = B // N_CORES      # 4
NT = V * PF // 128        # 16 n-tiles per batch, n-tile = 4 channels x 32 steps
NG = V // 4               # 16 groups of 4 channels for the MLP phase


def build_nc(b_loc=B_LOC, repeat=1):
    nc = bacc.Bacc(None, target_bir_lowering=False)

    x_in = nc.dram_tensor("x_s", [b_loc, L, V], F32, kind="ExternalInput")
    cent = nc.dram_tensor("centroids", [K, D], F32, kind="ExternalInput")
    enc_rhs_d = nc.dram_tensor("enc_rhs", [17, D], BF16, kind="ExternalInput")
    fc1_lhs_d = nc.dram_tensor("fc1_lhs", [128, H], BF16, kind="ExternalInput")
    fc1_b_d = nc.dram_tensor("fc1_b2", [128, 2], F32, kind="ExternalInput")
    fcm_lhs_d = nc.dram_tensor("fcm_lhs", [128, 2, 2 * H], BF16, kind="ExternalInput")
    fcm_b_d = nc.dram_tensor("fcm_b4", [128, 4], F32, kind="ExternalInput")
    fc2_rhs_d = nc.dram_tensor("fc2_rhs", [128, 4, PF], BF16, kind="ExternalInput")
    ct_d = nc.dram_tensor("ct", [D, K], F32R, kind="ExternalInput")
    c2aug_d = nc.dram_tensor("c2aug", [2, K], F32R, kind="ExternalInput")
    fuse_rhs_d = nc.dram_tensor("fuse_rhs", [D, D], F32R, kind="ExternalInput")
    dec_lhs_d = nc.dram_tensor("dec_lhs", [D, PL], BF16, kind="ExternalInput")

    out_s = nc.dram_tensor("out_s", [b_loc, PRED, V], F32, kind="ExternalOutput")
    logit_s = nc.dram_tensor("logit_s", [b_loc, V, PF, K], F32, kind="ExternalOutput")

    with tile.TileContext(nc) as tc:
        with (
            tc.tile_pool(name="const", bufs=1) as cpool,
            tc.tile_pool(name="work", bufs=2) as wpool,
            tc.tile_pool(name="mlp", bufs=4) as mpool,
            tc.tile_pool(name="vq", bufs=3) as vpool,
            tc.tile_pool(name="psE", bufs=2, space="PSUM") as psE,
            tc.tile_pool(name="psB", bufs=3, space="PSUM") as psB,
            tc.tile_pool(name="psS", bufs=3, space="PSUM") as psS,
            tc.tile_pool(name="dram", bufs=2, space="DRAM") as dpool,
        ):
            # ------------- constants -------------
            enc_rhs = cpool.tile_from(enc_rhs_d[:])
            fc1_lhs = cpool.tile_from(fc1_lhs_d[:])
            fc1_b = cpool.tile_from(fc1_b_d[:])
            fcm_lhs = cpool.tile_from(fcm_lhs_d[:])
            fcm_b = cpool.tile_from(fcm_b_d[:])
            fc2_rhs = cpool.tile_from(fc2_rhs_d[:])
            ct = cpool.tile_from(ct_d[:])
            c2aug = cpool.tile_from(c2aug_d[:])
            fuse_rhs = cpool.tile_from(fuse_rhs_d[:])
            dec_lhs = cpool.tile_from(dec_lhs_d[:])

            ident_bf = cpool.tile([128, 128], BF16)
            make_identity(nc, ident_bf[:])
            ident_f = cpool.tile([128, 128], F32)
            make_identity(nc, ident_f[:])
            ones_col_f = cpool.tile([128, 1], F32)
            nc.vector.memset(ones_col_f[:], 1.0)
            ones_col_bf = cpool.tile([128, 1], BF16)
            nc.vector.memset(ones_col_bf[:], 1.0)
            ones_row_bf = cpool.tile([1, 16], BF16)
            nc.vector.memset(ones_row_bf[:], 1.0)
            ones_sq = cpool.tile([64, 32], F32)
            nc.vector.memset(ones_sq[:], 1.0)
            ones_row64 = cpool.tile([1, 64], F32)
            nc.vector.memset(ones_row64[:], 1.0)
            zero_col = cpool.tile([128, 1], F32)
            nc.vector.memset(zero_col[:], 0.0)
            eps_col = cpool.tile([128, 1], F32)
            nc.vector.memset(eps_col[:], EPS)
            nc.const_aps.aps[(F32, 0.0)] = zero_col[:]
            nc.const_aps.aps[(F32, EPS)] = eps_col[:]

            for rep in range(repeat):
              for b in range(b_loc):
                # ==================================================
                # S1 RevIN stats: sum / sumsq over L via ones-matmul
                # ==================================================
                pstat = psS.tile([1, 512], F32, tag="s")
                for i in range(8):
                    xst = wpool.tile([128, 128], F32, tag="xst", bufs=3)
                    nc.sync.dma_start(xst[:, 0:64], x_in[b, i * 128:(i + 1) * 128, :])
                    nc.scalar.square(xst[:, 64:128], xst[:, 0:64])
                    nc.tensor.matmul(pstat[:, 0:128], ones_col_f[:], xst[:],
                                     start=(i == 0), stop=(i == 7))
                # srow: [mu(64) | sigma(64) | scratch(64)]
                srow = wpool.tile([1, 192], F32, tag="srow")
                nc.vector.tensor_scalar_mul(srow[:, 0:64], pstat[:, 0:64], 1.0 / L)
                # var = E[x^2] - mu^2 : scratch = -mu*mu, then += E[x2]
                nc.vector.scalar_tensor_tensor(
                    out=srow[:, 128:192], in0=srow[:, 0:64], scalar=-1.0,
                    in1=srow[:, 0:64], op0=OP.mult, op1=OP.mult)
                nc.vector.scalar_tensor_tensor(
                    out=srow[:, 128:192], in0=pstat[:, 64:128], scalar=1.0 / L,
                    in1=srow[:, 128:192], op0=OP.mult, op1=OP.add)
                nc.scalar.activation(srow[:, 64:128], srow[:, 128:192],
                                     AF.Sqrt, bias=EPS)
                # bf16 [mu | sigma] row for decoder psum-init
                musig_bf = wpool.tile([1, 128], BF16, tag="musig_bf")
                nc.vector.tensor_copy(musig_bf[:], srow[:, 0:128])
                # [mu|sigma] as columns via PE transpose
                pcol = psS.tile([128, 512], F32, tag="s")
                nc.tensor.transpose(pcol[:, 0:1], srow[:, 0:128], ident_f[0:1, 0:1])
                musig_col = wpool.tile([128, 1], F32, tag="musig_col")
                nc.vector.tensor_copy(musig_col[:], pcol[:, 0:1])
                sig_col = musig_col[64:128, :]

                # -mu replicated: [64 rep, 64 v] bf16 -> DRAM (outer product)
                pneg = psS.tile([128, 512], F32, tag="s")
                nc.tensor.matmul(pneg[0:64, 0:64], ones_row64[:],
                                 srow[:, 0:64], start=True, stop=True)
                negmu_rep = wpool.tile([64, 64], BF16, tag="negmu_rep")
                nc.vector.tensor_scalar(
                    out=negmu_rep[:], in0=pneg[0:64, 0:64],
                    scalar1=-1.0, scalar2=None, op0=OP.mult)
                negmu_d = dpool.tile([64, 64], BF16, tag="negmu_d")
                nc.sync.dma_start(negmu_d[:], negmu_rep[:])
                # sigma replicated per (v,f): [64 v, 32 f] f32 -> DRAM
                srep_sb = wpool.tile([64, 32], F32, tag="srep_sb")
                nc.vector.tensor_scalar_mul(srep_sb[:], ones_sq[:], sig_col)
                srep_d = dpool.tile([64, 32], F32, tag="srep_d")
                nc.sync.dma_start(srep_d[:], srep_sb[:])

                # ==================================================
                # S2 patch lhsT [17, (p v)] bf16; row16 = -mu_v
                # ==================================================
                xpatch = wpool.tile([17, PH * V], BF16, tag="xpatch")
                xsrc = x_in[b].rearrange("(p l) v -> l p v", l=PL)
                for q in range(4):
                    xpf = wpool.tile([16, PH * V // 4], F32, tag="xpatch_f",
                                     bufs=2)
                    nc.sync.dma_start(
                        xpf[:].rearrange("l (p v) -> l p v", v=V),
                        xsrc[:, q * 16:(q + 1) * 16, :])
                    nc.vector.tensor_copy(
                        xpatch[0:16, q * 1024:(q + 1) * 1024], xpf[:])
                nc.sync.dma_start(xpatch[16:17, :],
                                  negmu_d[:].rearrange("r v -> (r v)")[None, :])

                # ==================================================
                # S3 encoder GEMM + bn_stats + evac y (pre-LN, bf16)
                # ==================================================
                # paired layout: group 2*gp+h lives on partitions h*64..h*64+64
                sums_b = wpool.tile([128, 32], F32, tag="sums_b")
                sq_b = wpool.tile([128, 32], F32, tag="sq_b")
                y_all = mpool.tile([128, NG // 2, 512], BF16, tag="y_all", bufs=2)
                for gp in range(NG // 2):
                    pe = psB.tile([128, 512], F32, tag="b")
                    for h in range(2):
                        g = 2 * gp + h
                        for vq in range(4):
                            v = g * 4 + vq
                            nc.tensor.matmul(
                                pe[h * 64:(h + 1) * 64, vq * 128:(vq + 1) * 128],
                                xpatch[:, v::V], enc_rhs[:],
                                start=True, stop=True)
                    nc.vector.tensor_reduce(
                        sums_b[:, gp * 4:(gp + 1) * 4],
                        pe[:].rearrange("p (c d) -> p c d", d=128),
                        axis=AX.X, op=OP.add)
                    nc.scalar.copy(y_all[:, gp, :], pe[:])
                    y2 = wpool.tile([128, 512], BF16, tag="y2", bufs=3)
                    nc.scalar.square(y2[:], pe[:])
                    nc.vector.tensor_reduce(
                        sq_b[:, gp * 4:(gp + 1) * 4],
                        y2[:].rearrange("p (c d) -> p c d", d=128),
                        axis=AX.X, op=OP.add)

                # batched LN scalars as [128, 32] (paired columns)
                ms_b = wpool.tile([128, 128], F32, tag="ms_b")
                mean_b, rstd_b = ms_b[:, 0:32], ms_b[:, 32:64]
                mr_b, tmp_b = ms_b[:, 64:96], ms_b[:, 96:128]
                nc.vector.tensor_scalar_mul(mean_b, sums_b[:], 1.0 / D)
                # var = sq/D - mean^2 ; rstd = 1/sqrt(var + 1e-5)
                nc.vector.scalar_tensor_tensor(out=mr_b, in0=mean_b, scalar=-1.0,
                                               in1=mean_b, op0=OP.mult, op1=OP.mult)
                nc.vector.scalar_tensor_tensor(out=tmp_b, in0=sq_b[:],
                                               scalar=1.0 / D,
                                               in1=mr_b, op0=OP.mult, op1=OP.add)
                nc.scalar.activation(tmp_b, tmp_b, AF.Sqrt, bias=1e-5)
                nc.vector.reciprocal(rstd_b, tmp_b)
                nc.vector.scalar_tensor_tensor(out=mr_b, in0=mean_b, scalar=-1.0,
                                               in1=rstd_b, op0=OP.mult, op1=OP.mult)

                # ==================================================
                # S4 LN normalize per channel: X0a = y*rstd - mean*rstd
                # ==================================================
                x0a = mpool.tile([128, NG // 2, 512], BF16, tag="x0a", bufs=2)
                engines = [nc.vector, nc.scalar]
                for v in range(V):
                    gp, h, vq = v // 8, (v // 4) % 2, v % 4
                    hs = slice(h * 64, (h + 1) * 64)
                    ysl = y_all[hs, gp, vq * 128:(vq + 1) * 128]
                    xsl = x0a[hs, gp, vq * 128:(vq + 1) * 128]
                    sc = rstd_b[hs, gp * 4 + vq:gp * 4 + vq + 1]
                    bi = mr_b[hs, gp * 4 + vq:gp * 4 + vq + 1]
                    eng = engines[v % 2]
                    if eng is nc.scalar:
                        nc.scalar.activation(xsl, ysl, AF.Identity,
                                             scale=sc, bias=bi)
                    else:
                        eng.scalar_tensor_tensor(
                            out=xsl, in0=ysl, scalar=sc,
                            in1=bi.to_broadcast([64, 128]),
                            op0=OP.mult, op1=OP.add)

                # ==================================================
                # S5 MLP: fc1 -> fcm -> fc2, z_pD2 = 2*z_p [d, (v f)]
                # ==================================================
                zpd2 = vpool.tile([128, NT * 128], F32R, tag="zpd2", bufs=2)
                zsq = vpool.tile([128, NT * 128], BF16, tag="zsq", bufs=1)
                for g in range(NG):
                    gp, h = g // 2, g % 2
                    hs = slice(h * 64, (h + 1) * 64)
                    o1 = []
                    for j in range(2):
                        p1 = psB.tile([128, 512], F32, tag="b")
                        nc.tensor.matmul(p1[:], fc1_lhs[hs, j * 128:(j + 1) * 128],
                                         x0a[hs, gp, :], start=True, stop=True)
                        t1 = mpool.tile([128, 512], BF16, tag="o1", bufs=4)
                        nc.scalar.activation(t1[:], p1[:], AF.Relu,
                                             bias=fc1_b[:, j:j + 1])
                        o1.append(t1)
                    o2 = []
                    for j2 in range(4):
                        p2 = psB.tile([128, 512], F32, tag="b")
                        for kt in range(2):
                            nc.tensor.matmul(
                                p2[:],
                                fcm_lhs[:, kt, j2 * 128:(j2 + 1) * 128],
                                o1[kt][:], start=(kt == 0), stop=(kt == 1))
                        t2 = mpool.tile([128, 512], BF16, tag="o2", bufs=8)
                        if j2 % 2 == 0:
                            nc.scalar.activation(t2[:], p2[:], AF.Relu,
                                                 bias=fcm_b[:, j2:j2 + 1])
                        else:
                            nc.vector.tensor_scalar(
                                out=t2[:], in0=p2[:], scalar1=fcm_b[:, j2:j2 + 1],
                                scalar2=0.0, op0=OP.add, op1=OP.max)
                        o2.append(t2)
                    pz = psS.tile([128, 512], F32, tag="s")
                    for vq in range(4):
                        v = g * 4 + vq
                        for kt in range(4):
                            nc.tensor.matmul(
                                pz[:, vq * 32:(vq + 1) * 32],
                                o2[kt][:, vq * 128:(vq + 1) * 128],
                                fc2_rhs[:, kt, :], start=(kt == 0), stop=(kt == 3))
                    # evac: z_pD2 = 2*z_p (f32r) ; zsq = (2 z_p)^2 bf16
                    nc.scalar.activation(zpd2[:, g * 128:(g + 1) * 128],
                                         pz[:, 0:128], AF.Copy, scale=2.0)
                    nc.scalar.square(zsq[:, g * 128:(g + 1) * 128],
                                     zpd2[:, g * 128:(g + 1) * 128])

                # z2 row: z2aug[0,:] = -0.25 * ones^T @ zsq = -|z_p|^2
                z2aug = vpool.tile([2, NT * 128], F32R, tag="z2aug", bufs=1)
                nc.vector.memset(z2aug[:, :].bitcast(F32), 1.0)
                for q in range(4):
                    pq = psS.tile([1, 512], F32, tag="s")
                    nc.tensor.matmul(pq[:], ones_col_bf[:],
                                     zsq[:, q * 512:(q + 1) * 512],
                                     start=True, stop=True)
                    nc.vector.tensor_scalar_mul(
                        z2aug[0:1, q * 512:(q + 1) * 512], pq[:], -0.25)

                # ==================================================
                # S6 VQ distances + logits out + top-5
                # ==================================================
                maxv_b = vpool.tile([128, NT * 8], F32, tag="maxv", bufs=2)
                maxi_b = vpool.tile([128, NT * 8], U32, tag="maxi", bufs=2)
                logits_t = []
                for t in range(NT):
                    lg = vpool.tile([128, K], F32, tag="lg", bufs=3)
                    for c in range(2):
                        pd = psE.tile([128, 512], F32, tag="e")
                        nc.tensor.matmul(
                            pd[:], z2aug[:, t * 128:(t + 1) * 128],
                            c2aug[:, c * 512:(c + 1) * 512],
                            start=True, stop=False)
                        nc.tensor.matmul(
                            pd[:], zpd2[:, t * 128:(t + 1) * 128],
                            ct[:, c * 512:(c + 1) * 512],
                            start=False, stop=True)
                        nc.scalar.activation(lg[:, c * 512:(c + 1) * 512], pd[:],
                                             AF.Copy, scale=1.0 / TEMP)
                    nc.sync.dma_start(
                        logit_s[b, 4 * t:4 * t + 4].rearrange("v f k -> (v f) k"),
                        lg[:])
                    nc.vector.max(out=maxv_b[:, t * 8:(t + 1) * 8], in_=lg[:])
                    nc.vector.max_index(out=maxi_b[:, t * 8:(t + 1) * 8],
                                        in_max=maxv_b[:, t * 8:(t + 1) * 8],
                                        in_values=lg[:])

                # batched top-5 softmax -> w5_b [128, t, 5] f32
                w5_b = vpool.tile([128, NT * 8], F32, tag="w5", bufs=2)
                den_b = vpool.tile([128, 2 * NT], F32, tag="den", bufs=2)
                mv = maxv_b[:].rearrange("p (t k) -> p t k", k=8)
                w5v = w5_b[:].rearrange("p (t k) -> p t k", k=8)
                nc.vector.tensor_tensor(
                    w5v[:, :, 0:5], mv[:, :, 0:5],
                    mv[:, :, 0:1].to_broadcast([128, NT, 5]), op=OP.subtract)
                nc.scalar.activation(w5v[:, :, 0:5], w5v[:, :, 0:5], AF.Exp)
                nc.vector.tensor_reduce(den_b[:, 0:NT], w5v[:, :, 0:5],
                                        axis=AX.X, op=OP.add)
                nc.vector.reciprocal(den_b[:, NT:2 * NT], den_b[:, 0:NT])
                nc.vector.tensor_tensor(
                    w5v[:, :, 0:5], w5v[:, :, 0:5],
                    den_b[:, NT:2 * NT, None].to_broadcast([128, NT, 5]),
                    op=OP.mult)

                # ==================================================
                # S7 fuse + gather + weighted sum + fusion LN stats
                # ==================================================
                s6f_b = vpool.tile([128, NT * 2], F32, tag="s6f", bufs=2)
                acc_b = vpool.tile([128, NT * 128], F32, tag="acc", bufs=1)
                for t in range(NT):
                    acc = acc_b[:, t * 128:(t + 1) * 128]
                    pf = psS.tile([128, 512], F32, tag="s")
                    nc.tensor.matmul(pf[:, 0:128],
                                     zpd2[:, t * 128:(t + 1) * 128],
                                     fuse_rhs[:], start=True, stop=True)
                    nc.scalar.activation(acc, pf[:, 0:128], AF.Relu, scale=0.5)
                    for slot in range(TOPK):
                        gt = vpool.tile([128, 128], F32, tag="gath", bufs=10)
                        nc.gpsimd.indirect_dma_start(
                            out=gt[:], out_offset=None, in_=cent[:],
                            in_offset=IndirectOffsetOnAxis(
                                ap=maxi_b[:, t * 8 + slot:t * 8 + slot + 1],
                                axis=0))
                        nc.vector.scalar_tensor_tensor(
                            out=acc, in0=gt[:],
                            scalar=w5_b[:, t * 8 + slot:t * 8 + slot + 1],
                            in1=acc, op0=OP.mult, op1=OP.add)
                    s6f = wpool.tile([128, 6], F32, tag="s6f_t", bufs=4)
                    nc.vector.bn_stats(s6f[:], acc)
                    nc.vector.bn_aggr(
                        s6f_b[:, t * 2:(t + 1) * 2], s6f[:])

                # batched fusion-LN scalars; fold sigma into scale
                srep_b = wpool.tile([128, NT], F32, tag="srep_b")
                nc.sync.dma_start(
                    srep_b[:], srep_d[:].rearrange("(t a) f -> (a f) t", a=4))
                msf_b = wpool.tile([128, 2 * NT], F32, tag="msf_b")
                s6fv = s6f_b[:].rearrange("p (t s) -> p t s", s=2)
                fsc_b, fbi_b = msf_b[:, 0:NT], msf_b[:, NT:2 * NT]
                nc.scalar.activation(fsc_b, s6fv[:, :, 1], AF.Sqrt, bias=1e-5)
                nc.vector.reciprocal(fsc_b, fsc_b)
                nc.vector.tensor_tensor(fsc_b, fsc_b, srep_b[:], op=OP.mult)
                nc.vector.scalar_tensor_tensor(out=fbi_b, in0=s6fv[:, :, 0],
                                               scalar=-1.0, in1=fsc_b,
                                               op0=OP.mult, op1=OP.mult)

                # ==================================================
                # S8 fusion-LN normalize (bf16) + transpose -> zfT
                # ==================================================
                zft = vpool.tile([128, NT * 128], BF16, tag="zft", bufs=2)
                for t in range(NT):
                    zfs = wpool.tile([128, 128], BF16, tag="zfs", bufs=4)
                    nc.scalar.activation(zfs[:], acc_b[:, t * 128:(t + 1) * 128],
                                         AF.Identity,
                                         scale=fsc_b[:, t:t + 1],
                                         bias=fbi_b[:, t:t + 1])
                    pt = psS.tile([128, 512], F32, tag="s")
                    ptb = pt[:, 0:64].bitcast(BF16)
                    nc.tensor.transpose(ptb, zfs[:], ident_bf[:])
                    nc.vector.tensor_copy(zft[:, t * 128:(t + 1) * 128], ptb)

                # ==================================================
                # S9 decoder: psum preloaded with mu, [t, v] layout out
                # ==================================================
                prt = psB.tile([128, 512], F32, tag="b")
                for f in range(PF):
                    sec = prt[0:64, f * 16:(f + 1) * 16]
                    nc.tensor.matmul(sec, musig_bf[:, 0:64], ones_row_bf[:],
                                     start=True, stop=False)
                    nc.tensor.matmul(sec, zft[:, f::PF], dec_lhs[:],
                                     start=False, stop=True)
                recT = wpool.tile([64, 512], F32, tag="recT", bufs=2)
                nc.scalar.copy(recT[:], prt[0:64, :])
                for blk in range(4):
                    pdc = psS.tile([128, 512], F32, tag="s")
                    nc.tensor.transpose(pdc[:, 0:64],
                                        recT[:, blk * 128:(blk + 1) * 128],
                                        ident_f[0:64, 0:64])
                    rec = wpool.tile([128, 64], F32, tag="rec", bufs=3)
                    nc.scalar.copy(rec[:], pdc[:, 0:64])
                    nc.sync.dma_start(
                        out_s[b, blk * 128:(blk + 1) * 128, :], rec[:])

    nc.compile()
    nc.compile()
    return nc


def _prep_weights(ip):
    """Host-side weight prep (shared across cores)."""
    bf = ml_dtypes.bfloat16
    enc_w = ip["enc_w"].astype(np.float32)          # [16, 128]
    enc_b = ip["enc_b"].astype(np.float32)          # [128]
    assert np.abs(enc_b).max() == 0.0, "enc_b fold not emitted"
    s_w = enc_w.sum(axis=0)                          # [128]
    enc_rhs = np.concatenate([enc_w, s_w[None, :]], 0).astype(bf)   # [17,128]

    ln_w, ln_b = ip["ln_w"], ip["ln_b"]
    assert np.allclose(ln_w, 1.0) and np.abs(ln_b).max() == 0.0, \
        "general ln_w/ln_b path not emitted"

    fc1_lhs = np.vstack([ip["fc1_w"], ip["fc1_w"]]).astype(bf)  # [128,256]
    fc1_b2 = np.ascontiguousarray(
        ip["fc1_b"].astype(np.float32).reshape(2, 128).T)   # [128, 2]
    fcm_lhs = np.ascontiguousarray(
        ip["fcm_w"].astype(bf).reshape(2, 128, 512).transpose(1, 0, 2))
    fcm_b4 = np.ascontiguousarray(
        ip["fcm_b"].astype(np.float32).reshape(4, 128).T)   # [128, 4]
    fc2_rhs = np.ascontiguousarray(
        ip["fc2_w"].astype(bf).reshape(4, 128, PF).transpose(1, 0, 2))
    assert np.abs(ip["fc2_b"]).max() == 0.0, "fc2_b fold not emitted"

    cent = ip["centroids"].astype(np.float32)        # [1024, 128]
    ct = np.ascontiguousarray(cent.T)                # [128, 1024]
    c2 = (cent.astype(np.float64) ** 2).sum(1).astype(np.float32)
    c2aug = np.stack([np.ones(K, np.float32), -c2])  # [2, 1024]

    fuse_rhs = ip["fuse_w"].astype(np.float32)       # [128, 128]
    assert np.abs(ip["fuse_b"]).max() == 0.0, "fuse_b fold not emitted"
    assert np.allclose(ip["fln_w"], 1.0) and np.abs(ip["fln_b"]).max() == 0.0, \
        "general fln path not emitted"
    dec_lhs = ip["dec_w"].astype(bf)                 # [128, 16]
    assert np.abs(ip["dec_b"]).max() == 0.0, "dec_b fold not emitted"

    return {
        "centroids": cent, "enc_rhs": enc_rhs, "fc1_lhs": fc1_lhs,
        "fc1_b2": fc1_b2, "fcm_lhs": fcm_lhs, "fcm_b4": fcm_b4,
        "fc2_rhs": fc2_rhs, "ct": ct, "c2aug": c2aug,
        "fuse_rhs": fuse_rhs, "dec_lhs": dec_lhs,
    }


_NC_CACHE = {}


def kernel(**inputs):
    from concourse.bass_utils import run_bass_kernel_spmd

    if "nc" not in _NC_CACHE:
        _NC_CACHE["nc"] = build_nc()
    nc = _NC_CACHE["nc"]

    w = _prep_weights(inputs)
    x = np.ascontiguousarray(inputs["x"].astype(np.float32))
    in_maps = []
    for c in range(N_CORES):
        m = dict(w)
        m["x_s"] = np.ascontiguousarray(x[c * B_LOC:(c + 1) * B_LOC])
        in_maps.append(m)

    res = run_bass_kernel_spmd(nc, in_maps, core_ids=list(range(N_CORES)))
    outs = np.concatenate([r["out_s"] for r in res.results], axis=0)
    logits = np.concatenate([r["logit_s"] for r in res.results], axis=0)
    return outs, logits
